# revision 10
# baseline (speedup 1.0000x reference)
"""Trainium2 Bass kernel for DecoderLSTM with attention.

Sharding: data-parallel over batch (B=32 -> 4 rows/core) for the
recurrence; vocab-split (V=50000 -> 6250/core) for the fc_out
projection, with an AllGather of the (transposed, bf16) hidden states
between the two phases.

Self-contained: hardcodes all shapes; host-side prep is pure numpy.
"""

import numpy as np
import ml_dtypes

import concourse.bass as bass
import concourse.bacc as bacc
import concourse.mybir as mybir
import concourse.tile as tile
from concourse.bass import ts
from concourse.bass_utils import run_bass_kernel_spmd

BF16 = ml_dtypes.bfloat16

# Problem shapes (fixed).
B, T, P, H, V = 32, 20, 196, 512, 50000
E = A = 512
NCORES = 8
BL = B // NCORES            # 4 batch rows per core
VL = V // NCORES            # 6250 vocab rows per core
S = T - 1                   # 19 recurrence steps
R = S * BL                  # 76 (t, b_local) rows per core
ROWS = S * B                # 608 rows after allgather

HC = H // 128               # 4 chunks of hidden dim
AC = A // 128               # 4 chunks of attention dim
GC = (4 * H) // 128         # 16 chunks of gate dim
KC = (2 * H) // 128         # 8 contraction chunks for [ctx; h]
P0, P1 = 128, P - 128       # attention position chunks: 128 + 68

_CACHED = {}


def _dt(np_arr, dtype):
    return np.ascontiguousarray(np_arr).astype(dtype)


def _build_program():
    nc = bacc.Bacc()
    f32 = mybir.dt.float32
    bf16 = mybir.dt.bfloat16

    # ---- I/O ----
    encT = nc.declare_dram_parameter("encT", [H, BL * P], bf16, isOutput=False)
    encP = nc.declare_dram_parameter("encP", [2, 128, BL, H], bf16, isOutput=False)
    WeT = nc.declare_dram_parameter("WeT", [H, A], bf16, isOutput=False)
    WdT = nc.declare_dram_parameter("WdT", [H, A], bf16, isOutput=False)
    WchT = nc.declare_dram_parameter("WchT", [2 * H, 4 * H], bf16, isOutput=False)
    WieT = nc.declare_dram_parameter("WieT", [E, 4 * H], bf16, isOutput=False)
    WoT = nc.declare_dram_parameter("WoT", [H, VL], bf16, isOutput=False)
    embT = nc.declare_dram_parameter("embT", [E, R], bf16, isOutput=False)
    w_e_d = nc.declare_dram_parameter("w_e", [128, AC], bf16, isOutput=False)
    bias_a_d = nc.declare_dram_parameter("bias_a", [128, AC], f32, isOutput=False)
    bias_g_d = nc.declare_dram_parameter("bias_g", [128, GC], f32, isOutput=False)
    b_e_d = nc.declare_dram_parameter("b_e", [1, 1], f32, isOutput=False)

    out_v = nc.declare_dram_parameter("out_v", [ROWS, VL], f32, isOutput=True)
    attw_o = nc.declare_dram_parameter("attw_o", [1, BL * P], f32, isOutput=True)

    ag_in = nc.dram_tensor("ag_in", [H, R], bf16)
    ag_out = nc.dram_tensor("ag_out", [NCORES * H, R], bf16, addr_space="Shared")

    Tanh = mybir.ActivationFunctionType.Tanh
    Sigmoid = mybir.ActivationFunctionType.Sigmoid
    Exp = mybir.ActivationFunctionType.Exp
    Copy = mybir.ActivationFunctionType.Copy
    Identity = mybir.ActivationFunctionType.Identity

    def bcast(ap, n):
        """Broadcast an AP along a trailing free dim of size n (step 0)."""
        return bass.AP(tensor=ap.tensor, offset=ap.offset, ap=[*ap.ap, [0, n]])

    with tile.TileContext(nc) as tc:
        with tc.tile_pool(name="singles", bufs=1) as singles:
            # ---- resident tiles + input DMAs ----
            WeT_sb = singles.tile([128, HC, A], bf16)
            WdT_sb = singles.tile([128, HC, A], bf16)
            WchT_sb = singles.tile([128, KC, 4 * H], bf16)
            WieT_sb = singles.tile([128, HC, 4 * H], bf16)
            encT_sb = singles.tile([128, HC, BL * P], bf16)
            encP_sb = singles.tile([128, 2, BL, H], bf16)
            embT_sb = singles.tile([128, HC, R], bf16)
            w_e_sb = singles.tile([128, AC], bf16)
            bias_a_sb = singles.tile([128, AC], f32)
            bias_g_sb = singles.tile([128, GC], f32)
            b_e_sb = singles.tile([1, 1], f32)
            one_bf = singles.tile([1, 1], bf16)
            WoT_sb = singles.tile([128, HC, VL], bf16)

            enc_projT = singles.tile([128, AC, BL, P], f32)
            emb_preT = singles.tile([128, GC, R], f32)
            hist = singles.tile([128, HC, 4 * (S + 1)], bf16)
            c_st = singles.tile([128, HC, BL], f32)
            HT_sb = singles.tile([128, HC, ROWS], bf16)

            for k in range(HC):
                nc.sync.dma_start(
                    out=WeT_sb[:, k, :],
                    in_=WeT.rearrange("(c p) a -> c p a", p=128)[k],
                )
                nc.sync.dma_start(
                    out=WdT_sb[:, k, :],
                    in_=WdT.rearrange("(c p) a -> c p a", p=128)[k],
                )
                nc.sync.dma_start(
                    out=encT_sb[:, k, :],
                    in_=encT.rearrange("(c p) n -> c p n", p=128)[k],
                )
                nc.sync.dma_start(
                    out=embT_sb[:, k, :],
                    in_=embT.rearrange("(c p) n -> c p n", p=128)[k],
                )
                nc.sync.dma_start(
                    out=WieT_sb[:, k, :],
                    in_=WieT.rearrange("(c p) g -> c p g", p=128)[k],
                )
            for k in range(KC):
                nc.sync.dma_start(
                    out=WchT_sb[:, k, :],
                    in_=WchT.rearrange("(c p) g -> c p g", p=128)[k],
                )
            for cc in range(2):
                nc.sync.dma_start(out=encP_sb[:, cc, :, :], in_=encP[cc, :, :, :])
            nc.sync.dma_start(out=w_e_sb, in_=w_e_d[:, :])
            nc.sync.dma_start(out=bias_a_sb, in_=bias_a_d[:, :])
            nc.sync.dma_start(out=bias_g_sb, in_=bias_g_d[:, :])
            nc.sync.dma_start(out=b_e_sb, in_=b_e_d[:, :])
            nc.vector.memset(one_bf, 1.0)
            for k in range(HC):
                nc.sync.dma_start(
                    out=WoT_sb[:, k, :],
                    in_=WoT.rearrange("(c p) v -> c p v", p=128)[k],
                )

            nc.vector.memset(hist[:, :, 0:BL], 0.0)
            nc.vector.memset(c_st, 0.0)

            # ---- precompute: enc_proj (+bias_a) and emb_pre (+bias_g) ----
            NSPL = [(0, 512), (512, BL * P - 512)]  # 784 = 512 + 272
            with tc.tile_pool(name="pre_psum", bufs=2, space="PSUM") as pp, \
                 tc.tile_pool(name="pre_sb", bufs=2) as _psb:
                for cc in range(AC):
                    for n0, nn in NSPL:
                        pe_ps = pp.tile([128, 512], f32, tag="pe")
                        for k in range(HC):
                            nc.tensor.matmul(
                                pe_ps[:, 0:nn],
                                WeT_sb[:, k, ts(cc, 128)],
                                encT_sb[:, k, n0:n0 + nn],
                                start=(k == 0),
                                stop=(k == HC - 1),
                            )
                        nc.scalar.activation(
                            out=enc_projT.rearrange("p c b q -> p c (b q)")[:, cc, n0:n0 + nn],
                            in_=pe_ps[:, 0:nn],
                            func=Identity,
                            bias=bias_a_sb[:, cc:cc + 1],
                        )
                for m in range(GC):
                    em_ps = pp.tile([128, R], f32, tag="em")
                    for k in range(HC):
                        nc.tensor.matmul(
                            em_ps,
                            WieT_sb[:, k, ts(m, 128)],
                            embT_sb[:, k, :],
                            start=(k == 0),
                            stop=(k == HC - 1),
                        )
                    nc.scalar.activation(
                        out=emb_preT[:, m, :],
                        in_=em_ps,
                        func=Identity,
                        bias=bias_g_sb[:, m:m + 1],
                    )

            # ---- recurrence ----
            with tc.tile_pool(name="st_psum", bufs=1, space="PSUM") as sp1, \
                 tc.tile_pool(name="st_psum2", bufs=2, space="PSUM") as sp2, \
                 tc.tile_pool(name="st_sb", bufs=2) as ssb:
                for t in range(S):
                    hcur = hist[:, :, ts(t, BL)]
                    hnxt = hist[:, :, ts(t + 1, BL)]

                    # dp = h @ Wd.T  -> [a, b] chunks
                    dp_ps = sp1.tile([128, AC, BL], f32, tag="dp")
                    for m in range(AC):
                        for k in range(HC):
                            nc.tensor.matmul(
                                dp_ps[:, m, :],
                                WdT_sb[:, k, ts(m, 128)],
                                hcur[:, k, :],
                                start=(m == 0 and k == 0),
                                stop=(m == AC - 1 and k == HC - 1),
                            )
                    dpT = ssb.tile([128, AC, BL], f32, tag="dpT")
                    nc.scalar.copy(dpT, dp_ps)

                    # gates (h part) early: overlaps attention on ACT/DVE
                    g_ps = sp1.tile([128, GC, BL], f32, tag="g")
                    for m in range(GC):
                        for k in range(HC):
                            nc.tensor.matmul(
                                g_ps[:, m, :],
                                WchT_sb[:, AC + k, ts(m, 128)],
                                hcur[:, k, :],
                                start=(m == 0 and k == 0),
                                stop=False,
                            )

                    # tanh(enc_proj + dp) -> bf16
                    tanh_bf = []
                    for cc in range(AC):
                        targ = ssb.tile([128, BL, P], f32, tag="targ")
                        nc.vector.tensor_add(
                            targ, enc_projT[:, cc, :, :], bcast(dpT[:, cc, :], P)
                        )
                        tbf = ssb.tile([128, BL, P], bf16, tag=f"tbf{cc % 2}")
                        nc.scalar.activation(out=tbf, in_=targ, func=Tanh)
                        tanh_bf.append(tbf)

                    # scores + exp + per-b denom
                    exp_sc = ssb.tile([1, BL, P], f32, tag="exp")
                    denom = ssb.tile([1, BL], f32, tag="den")
                    for b in range(BL):
                        sc_ps = sp2.tile([1, P], f32, tag="sc")
                        for cc in range(AC):
                            nc.tensor.matmul(
                                sc_ps,
                                w_e_sb[:, cc:cc + 1],
                                tanh_bf[cc][:, b, :],
                                start=(cc == 0),
                                stop=(cc == AC - 1),
                            )
                        nc.scalar.activation(
                            out=exp_sc[:, b, :],
                            in_=sc_ps,
                            func=Exp,
                            bias=b_e_sb,
                            accum_out=denom[:, b:b + 1],
                        )
                    rden = ssb.tile([1, BL], f32, tag="rden")
                    nc.vector.reciprocal(rden, denom)
                    attw = ssb.tile([1, BL, P], f32, tag="attw")
                    nc.vector.tensor_mul(attw, exp_sc, bcast(rden, P))
                    attw_bf = ssb.tile([1, BL, P], bf16, tag="attwbf")
                    nc.vector.tensor_copy(attw_bf, attw)
                    if t == S - 1:
                        nc.sync.dma_start(out=attw_o[:, :], in_=attw.rearrange("p b q -> p (b q)"))

                    # context: broadcast attw across partitions, then
                    # per-(b, d-chunk) matvecs against encP
                    ctx_ps = sp1.tile([128, HC, BL], f32, tag="ctx")
                    attP = ssb.tile([128, 2, BL], bf16, tag="attP")
                    for b in range(BL):
                        aw0_ps = sp1.tile([128, 1], f32, tag="aw0")
                        aw1_ps = sp1.tile([P1, 1], f32, tag="aw1")
                        nc.tensor.matmul(
                            aw0_ps, attw_bf[:, b, 0:P0], one_bf,
                            start=True, stop=True,
                        )
                        nc.tensor.matmul(
                            aw1_ps, attw_bf[:, b, P0:P], one_bf,
                            start=True, stop=True,
                        )
                        nc.vector.tensor_copy(attP[:, 0, b:b + 1], aw0_ps)
                        nc.vector.tensor_copy(attP[0:P1, 1, b:b + 1], aw1_ps)
                        for m in range(HC):
                            nc.tensor.matmul(
                                ctx_ps[:, m, b:b + 1],
                                encP_sb[:, 0, b, ts(m, 128)],
                                attP[:, 0, b:b + 1],
                                start=(b == 0 and m == 0),
                                stop=False,
                            )
                            nc.tensor.matmul(
                                ctx_ps[:, m, b:b + 1],
                                encP_sb[0:P1, 1, b, ts(m, 128)],
                                attP[0:P1, 1, b:b + 1],
                                start=False,
                                stop=(b == BL - 1 and m == HC - 1),
                            )
                    ctx_bf = ssb.tile([128, HC, BL], bf16, tag="ctxbf")
                    nc.vector.tensor_copy(ctx_bf, ctx_ps)

                    # gates (ctx part)
                    for m in range(GC):
                        for k in range(AC):
                            nc.tensor.matmul(
                                g_ps[:, m, :],
                                WchT_sb[:, k, ts(m, 128)],
                                ctx_bf[:, k, :],
                                start=False,
                                stop=(m == GC - 1 and k == AC - 1),
                            )

                    # gates += emb_pre; LSTM cell update
                    gsb = ssb.tile([128, GC, BL], f32, tag="gsb")
                    nc.vector.tensor_add(gsb, g_ps, emb_preT[:, :, ts(t, BL)])
                    sif = ssb.tile([128, 2 * HC, BL], f32, tag="sif")
                    nc.scalar.activation(out=sif, in_=gsb[:, 0:2 * HC, :], func=Sigmoid)
                    tg = ssb.tile([128, HC, BL], f32, tag="tg")
                    nc.scalar.activation(out=tg, in_=gsb[:, 2 * HC:3 * HC, :], func=Tanh)
                    so = ssb.tile([128, HC, BL], f32, tag="so")
                    nc.scalar.activation(out=so, in_=gsb[:, 3 * HC:4 * HC, :], func=Sigmoid)
                    t1 = ssb.tile([128, HC, BL], f32, tag="t1")
                    nc.vector.tensor_mul(t1, sif[:, HC:2 * HC, :], c_st)
                    t2 = ssb.tile([128, HC, BL], f32, tag="t2")
                    nc.vector.tensor_mul(t2, sif[:, 0:HC, :], tg)
                    nc.vector.tensor_add(c_st, t1, t2)
                    tc_t = ssb.tile([128, HC, BL], f32, tag="tct")
                    nc.scalar.activation(out=tc_t, in_=c_st, func=Tanh)
                    nc.vector.tensor_mul(hnxt, so, tc_t)

            # ---- allgather h ----
            for k in range(HC):
                nc.sync.dma_start(
                    out=ag_in.rearrange("(c p) r -> c p r", p=128)[k],
                    in_=hist[:, k, BL:],
                )
            nc.gpsimd.collective_compute(
                "AllGather",
                mybir.AluOpType.bypass,
                ins=[ag_in[:, :]],
                outs=[ag_out[:, :]],
                replica_groups=[list(range(NCORES))],
            )
            ag_v = ag_out.rearrange("(j c p) r -> j c p r", c=HC, p=128)
            for k in range(HC):
                for j in range(NCORES):
                    nc.sync.dma_start(
                        out=HT_sb[:, k, ts(j, R)], in_=ag_v[j, k]
                    )

            # ---- vocab projection: out[rows, v] = H_all @ WoT ----
            MTS = [(0, 128), (128, 128), (256, 128), (384, 128), (512, 96)]
            NV = [(i * 512, min(512, VL - i * 512)) for i in range((VL + 511) // 512)]
            with tc.tile_pool(name="v_psum", bufs=4, space="PSUM") as vp, \
                 tc.tile_pool(name="v_sb", bufs=4) as vsb:
                for m0, mm in MTS:
                    for n0, nn in NV:
                        v_ps = vp.tile([128, 512], f32, tag="v")
                        for k in range(HC):
                            nc.tensor.matmul(
                                v_ps[0:mm, 0:nn],
                                HT_sb[:, k, m0:m0 + mm],
                                WoT_sb[:, k, n0:n0 + nn],
                                start=(k == 0),
                                stop=(k == HC - 1),
                            )
                        o_sb = vsb.tile([128, 512], f32, tag="o")
                        nc.vector.tensor_copy(o_sb[0:mm, 0:nn], v_ps[0:mm, 0:nn])
                        nc.sync.dma_start(
                            out=out_v[m0:m0 + mm, n0:n0 + nn], in_=o_sb[0:mm, 0:nn]
                        )

    nc.compile()
    return nc


def _prep_inputs(inputs):
    enc = np.asarray(inputs["encoder_outputs"], np.float32)      # [B, P, H]
    captions = np.asarray(inputs["captions"])
    embedding = np.asarray(inputs["embedding"], np.float32)      # [V, E]
    W_ih = np.asarray(inputs["W_ih"], np.float32)                # [4H, E+H]
    W_hh = np.asarray(inputs["W_hh"], np.float32)                # [4H, H]
    b_ih = np.asarray(inputs["b_ih"], np.float32)
    b_hh = np.asarray(inputs["b_hh"], np.float32)
    We = np.asarray(inputs["We"], np.float32)                    # [A, H]
    be = np.asarray(inputs["be"], np.float32)
    Wd = np.asarray(inputs["Wd"], np.float32)
    bd = np.asarray(inputs["bd"], np.float32)
    w_e = np.asarray(inputs["w_e"], np.float32)                  # [A]
    b_e = np.asarray(inputs["b_e"], np.float32)
    W_out = np.asarray(inputs["W_out"], np.float32)              # [V, H]

    emb = embedding[captions[:, :-1]]                            # [B, S, E]

    W_ie = W_ih[:, :E]
    W_ic = W_ih[:, E:]
    WchT = np.concatenate([W_ic.T, W_hh.T], axis=0)              # [2H, 4H]
    WieT = W_ie.T                                                # [E, 4H]
    bias_a = (be + bd).reshape(AC, 128).T                        # [128, AC]
    bias_g = (b_ih + b_hh).reshape(GC, 128).T                    # [128, GC]
    w_e_t = w_e.reshape(AC, 128).T                               # [128, AC]

    shared = {
        "WeT": _dt(We.T, BF16),
        "WdT": _dt(Wd.T, BF16),
        "WchT": _dt(WchT, BF16),
        "WieT": _dt(WieT, BF16),
        "w_e": _dt(w_e_t, BF16),
        "bias_a": _dt(bias_a, np.float32),
        "bias_g": _dt(bias_g, np.float32),
        "b_e": _dt(b_e.reshape(1, 1), np.float32),
    }

    in_maps = []
    for j in range(NCORES):
        bsl = slice(j * BL, (j + 1) * BL)
        enc_j = enc[bsl]                                          # [BL, P, H]
        encT = enc_j.reshape(BL * P, H).T                         # [H, BL*P]
        encP = np.zeros((2, 128, BL, H), np.float32)
        encP[0] = enc_j.transpose(1, 0, 2)[0:128]
        encP[1, 0:P1] = enc_j.transpose(1, 0, 2)[128:P]
        embT = emb[bsl].transpose(2, 1, 0).reshape(E, R)          # [E, (t,b)]
        WoT = W_out[j * VL:(j + 1) * VL].T                        # [H, VL]
        m = dict(shared)
        m.update({
            "encT": _dt(encT, BF16),
            "encP": _dt(encP, BF16),
            "embT": _dt(embT, BF16),
            "WoT": _dt(WoT, BF16),
        })
        in_maps.append(m)
    return in_maps


def kernel(**inputs):
    if "nc" not in _CACHED:
        _CACHED["nc"] = _build_program()
    nc = _CACHED["nc"]

    in_maps = _prep_inputs(inputs)
    res = run_bass_kernel_spmd(nc, in_maps, list(range(NCORES))).results

    outputs = np.zeros((B, T, V), np.float32)
    for j in range(NCORES):
        o = res[j]["out_v"].reshape(NCORES, S, BL, VL)            # (rank, t, b, v)
        outputs[:, 1:, j * VL:(j + 1) * VL] = (
            o.transpose(0, 2, 1, 3).reshape(B, S, VL)
        )
    b_out = np.asarray(inputs["b_out"], np.float32)
    if np.any(b_out):
        outputs[:, 1:, :] += b_out
    attw = np.concatenate(
        [res[j]["attw_o"].reshape(BL, P) for j in range(NCORES)], axis=0
    )
    return outputs, attw


# revision 16
# speedup vs baseline: 1.1202x; 1.1202x over previous
"""Trainium2 Bass kernel for DecoderLSTM with attention.

Sharding: data-parallel over batch (B=32 -> 4 rows/core) for the
recurrence; vocab-split (V=50000 -> 6250/core) for the fc_out
projection. Hidden states are AllGathered in 5 step-groups (bf16,
transposed) so the vocab projection overlaps the recurrence.

Key device-side tricks:
- all matmuls in bf16 (stationary weights get FWL), f32 elementwise
- sigmoid via 0.5*(1+tanh(x/2)) so the whole recurrence uses one ACT
  table set (exp_and_others: Tanh+Exp) -> no ~2.7us table swaps
- hist stores 2*h with Wd/W_hh/W_out pre-scaled by 0.5 on host
- attention softmax kept unnormalized; denominators reduced on the PE
  (ones-matvec) and folded into the context via a broadcast reciprocal
- gates accumulate in one PSUM bank: one start (zeroes the bank), one
  stop at the very last accumulating matmul

Self-contained: hardcodes all shapes; host-side prep is pure numpy.
"""

import numpy as np
import ml_dtypes

import concourse.bass as bass
import concourse.bacc as bacc
import concourse.mybir as mybir
import concourse.tile as tile
from concourse.bass import ts
from concourse.bass_utils import run_bass_kernel_spmd

BF16 = ml_dtypes.bfloat16

# Problem shapes (fixed).
B, T, P, H, V = 32, 20, 196, 512, 50000
E = A = 512
NCORES = 8
BL = B // NCORES            # 4 batch rows per core
VL = V // NCORES            # 6250 vocab rows per core
S = T - 1                   # 19 recurrence steps
R = S * BL                  # 76 (t, b_local) cols per core
ROWS = S * B                # 608 rows after allgather

HC = H // 128               # 4 chunks of hidden dim
AC = A // 128               # 4 chunks of attention dim
GC = (4 * H) // 128         # 16 chunks of gate dim
KC = (2 * H) // 128         # 8 contraction chunks for [ctx; h]
P0, P1 = 128, P - 128       # attention position chunks: 128 + 68

# Step-groups for the pipelined AllGather + vocab projection.
GROUPS = [(0, 4), (4, 4), (8, 4), (12, 4), (16, 3)]   # (first step, n steps)
NV = [(i * 512, min(512, VL - i * 512)) for i in range((VL + 511) // 512)]

_CACHED = {}


def _dt(np_arr, dtype):
    return np.ascontiguousarray(np_arr).astype(dtype)


def _build_program():
    nc = bacc.Bacc()
    f32 = mybir.dt.float32
    bf16 = mybir.dt.bfloat16

    # ---- I/O ----
    encT = nc.declare_dram_parameter("encT", [H, BL * P], bf16, isOutput=False)
    encP = nc.declare_dram_parameter("encP", [2, 128, BL, H], bf16, isOutput=False)
    WeT = nc.declare_dram_parameter("WeT", [H, A], bf16, isOutput=False)
    WdT = nc.declare_dram_parameter("WdT", [H, A], bf16, isOutput=False)
    WchT = nc.declare_dram_parameter("WchT", [2 * H, 4 * H], bf16, isOutput=False)
    WieT = nc.declare_dram_parameter("WieT", [E, 4 * H], bf16, isOutput=False)
    WoT = nc.declare_dram_parameter("WoT", [H, VL], bf16, isOutput=False)
    embT = nc.declare_dram_parameter("embT", [E, R], bf16, isOutput=False)
    w_e_d = nc.declare_dram_parameter("w_e", [128, AC], bf16, isOutput=False)
    bias_a_d = nc.declare_dram_parameter("bias_a", [128, AC], f32, isOutput=False)
    bias_g_d = nc.declare_dram_parameter("bias_g", [128, GC], f32, isOutput=False)
    b_e_d = nc.declare_dram_parameter("b_e", [1, 1], f32, isOutput=False)

    out_v = nc.declare_dram_parameter("out_v", [ROWS, VL], f32, isOutput=True)
    attw_o = nc.declare_dram_parameter("attw_o", [1, BL * P], f32, isOutput=True)

    ag_ins, ag_outs = [], []
    for gi, (_, sg) in enumerate(GROUPS):
        w = 4 * sg
        ag_ins.append(nc.dram_tensor(f"ag_in{gi}", [H, w], bf16))
        ag_outs.append(
            nc.dram_tensor(f"ag_out{gi}", [NCORES * H, w], bf16, addr_space="Shared")
        )

    Tanh = mybir.ActivationFunctionType.Tanh
    Exp = mybir.ActivationFunctionType.Exp
    Identity = mybir.ActivationFunctionType.Identity
    MULT = mybir.AluOpType.mult
    ADD = mybir.AluOpType.add

    def bcast(ap, n):
        """Broadcast an AP along a trailing free dim of size n (step 0)."""
        return bass.AP(tensor=ap.tensor, offset=ap.offset, ap=[*ap.ap, [0, n]])

    def bcast_mid(ap, n):
        """[p, x] -> [p, n, x] view with step 0 on the middle dim."""
        return bass.AP(
            tensor=ap.tensor, offset=ap.offset, ap=[ap.ap[0], [0, n], *ap.ap[1:]]
        )

    with tile.TileContext(nc) as tc:
        with tc.tile_pool(name="singles", bufs=1) as singles:
            # ---- resident tiles + input DMAs ----
            WeT_sb = singles.tile([128, HC, A], bf16)
            WdT_sb = singles.tile([128, HC, A], bf16)
            WchT_sb = singles.tile([128, KC, 4 * H], bf16)
            WieT_sb = singles.tile([128, HC, 4 * H], bf16)
            encT_sb = singles.tile([128, HC, BL * P], bf16)
            encP_sb = singles.tile([128, 2, BL, H], bf16)
            embT_sb = singles.tile([128, HC, R], bf16)
            w_e_sb = singles.tile([128, AC], bf16)
            bias_a_sb = singles.tile([128, AC], f32)
            bias_g_sb = singles.tile([128, GC], f32)
            b_e_sb = singles.tile([128, 1], f32)
            one_bf = singles.tile([1, 1], bf16)
            ones_f32 = singles.tile([1, 128], f32)
            WoT_sb = singles.tile([128, HC, VL], bf16)

            enc_projT = singles.tile([128, AC, BL, P], f32)
            emb_preT = singles.tile([128, GC, R], f32)
            hist = singles.tile([128, HC, 4 * (S + 1)], bf16)
            c_st = singles.tile([128, HC, BL], f32)
            HT_sb = singles.tile([128, HC, ROWS], bf16)

            for k in range(HC):
                nc.sync.dma_start(
                    out=WeT_sb[:, k, :],
                    in_=WeT.rearrange("(c p) a -> c p a", p=128)[k],
                )
                nc.sync.dma_start(
                    out=WdT_sb[:, k, :],
                    in_=WdT.rearrange("(c p) a -> c p a", p=128)[k],
                )
                nc.sync.dma_start(
                    out=encT_sb[:, k, :],
                    in_=encT.rearrange("(c p) n -> c p n", p=128)[k],
                )
                nc.sync.dma_start(
                    out=embT_sb[:, k, :],
                    in_=embT.rearrange("(c p) n -> c p n", p=128)[k],
                )
                nc.sync.dma_start(
                    out=WieT_sb[:, k, :],
                    in_=WieT.rearrange("(c p) g -> c p g", p=128)[k],
                )
            for k in range(KC):
                nc.sync.dma_start(
                    out=WchT_sb[:, k, :],
                    in_=WchT.rearrange("(c p) g -> c p g", p=128)[k],
                )
            for cc in range(2):
                nc.sync.dma_start(out=encP_sb[:, cc, :, :], in_=encP[cc, :, :, :])
            nc.sync.dma_start(out=w_e_sb, in_=w_e_d[:, :])
            nc.sync.dma_start(out=bias_a_sb, in_=bias_a_d[:, :])
            nc.sync.dma_start(out=bias_g_sb, in_=bias_g_d[:, :])
            nc.sync.dma_start(
                out=b_e_sb,
                in_=bass.AP(tensor=b_e_d, offset=0, ap=[[0, 128], [1, 1]]),
            )
            nc.vector.memset(one_bf, 1.0)
            nc.vector.memset(ones_f32, 1.0)
            for k in range(HC):
                nc.sync.dma_start(
                    out=WoT_sb[:, k, :],
                    in_=WoT.rearrange("(c p) v -> c p v", p=128)[k],
                )

            nc.vector.memset(hist[:, :, 0:BL], 0.0)
            nc.vector.memset(c_st, 0.0)

            # ---- precompute: enc_proj (+bias_a) and emb_pre (+bias_g) ----
            NSPL = [(0, 512), (512, BL * P - 512)]  # 784 = 512 + 272
            with tc.tile_pool(name="pre_psum", bufs=2, space="PSUM") as pp:
                for cc in range(AC):
                    for n0, nn in NSPL:
                        pe_ps = pp.tile([128, 512], f32, tag="pe")
                        for k in range(HC):
                            nc.tensor.matmul(
                                pe_ps[:, 0:nn],
                                WeT_sb[:, k, ts(cc, 128)],
                                encT_sb[:, k, n0:n0 + nn],
                                start=(k == 0),
                                stop=(k == HC - 1),
                            )
                        nc.scalar.activation(
                            out=enc_projT.rearrange("p c b q -> p c (b q)")[
                                :, cc, n0:n0 + nn
                            ],
                            in_=pe_ps[:, 0:nn],
                            func=Identity,
                            bias=bias_a_sb[:, cc:cc + 1],
                        )
                for m in range(GC):
                    em_ps = pp.tile([128, R], f32, tag="em")
                    for k in range(HC):
                        nc.tensor.matmul(
                            em_ps,
                            WieT_sb[:, k, ts(m, 128)],
                            embT_sb[:, k, :],
                            start=(k == 0),
                            stop=(k == HC - 1),
                        )
                    nc.scalar.activation(
                        out=emb_preT[:, m, :],
                        in_=em_ps,
                        func=Identity,
                        bias=bias_g_sb[:, m:m + 1],
                    )

            # ---- recurrence + pipelined allgather/vocab ----
            vocab_jobs = []

            with tc.tile_pool(name="st_psum", bufs=1, space="PSUM") as sp1, \
                 tc.tile_pool(name="st_psum2", bufs=2, space="PSUM") as sp2, \
                 tc.tile_pool(name="v_psum", bufs=2, space="PSUM") as vp, \
                 tc.tile_pool(name="st_sb", bufs=2) as ssb, \
                 tc.tile_pool(name="v_sb", bufs=3) as vsb:

                def emit_group_ag(gi):
                    s0, sg = GROUPS[gi]
                    w = 4 * sg
                    lo = 4 * (s0 + 1)
                    for k in range(HC):
                        nc.sync.dma_start(
                            out=ag_ins[gi].rearrange("(c p) r -> c p r", p=128)[k],
                            in_=hist[:, k, lo:lo + w],
                        )
                    nc.gpsimd.collective_compute(
                        "AllGather",
                        mybir.AluOpType.bypass,
                        ins=[ag_ins[gi][:, :]],
                        outs=[ag_outs[gi][:, :]],
                        replica_groups=[list(range(NCORES))],
                    )
                    ag_v = ag_outs[gi].rearrange("(j c p) r -> j c p r", c=HC, p=128)
                    for k in range(HC):
                        for j in range(NCORES):
                            nc.sync.dma_start(
                                out=HT_sb[
                                    :, k, 128 * gi + w * j:128 * gi + w * (j + 1)
                                ],
                                in_=ag_v[j, k],
                            )
                    for n0, nn in NV:
                        vocab_jobs.append((gi, n0, nn))

                def emit_vocab_jobs(count):
                    for _ in range(min(count, len(vocab_jobs))):
                        gi, n0, nn = vocab_jobs.pop(0)
                        m0 = 128 * gi
                        mm = 4 * GROUPS[gi][1] * NCORES
                        v_ps = vp.tile([128, 512], f32, tag="v")
                        for k in range(HC):
                            nc.tensor.matmul(
                                v_ps[0:mm, 0:nn],
                                HT_sb[:, k, m0:m0 + mm],
                                WoT_sb[:, k, n0:n0 + nn],
                                start=(k == 0),
                                stop=(k == HC - 1),
                            )
                        o_sb = vsb.tile([128, 512], f32, tag="o")
                        nc.vector.tensor_copy(o_sb[0:mm, 0:nn], v_ps[0:mm, 0:nn])
                        nc.sync.dma_start(
                            out=out_v[m0:m0 + mm, n0:n0 + nn], in_=o_sb[0:mm, 0:nn]
                        )

                for t in range(S):
                    hcur = hist[:, :, ts(t, BL)]
                    hnxt = hist[:, :, ts(t + 1, BL)]

                    # dp = (2h) @ (0.5*Wd).T  -> [a, b] chunks
                    dp_ps = sp1.tile([128, AC, BL], f32, tag="dp")
                    for m in range(AC):
                        for k in range(HC):
                            nc.tensor.matmul(
                                dp_ps[:, m, :],
                                WdT_sb[:, k, ts(m, 128)],
                                hcur[:, k, :],
                                start=(m == 0 and k == 0),
                                stop=(m == AC - 1 and k == HC - 1),
                            )
                    dpT = ssb.tile([128, AC, BL], f32, tag="dpT")
                    nc.vector.tensor_copy(dpT, dp_ps)

                    # gates (h part) early: fills PE while ACT does tanh
                    g_ps = sp1.tile([128, GC, BL], f32, tag="g")
                    for m in range(GC):
                        for k in range(HC):
                            nc.tensor.matmul(
                                g_ps[:, m, :],
                                WchT_sb[:, AC + k, ts(m, 128)],
                                hcur[:, k, :],
                                start=(m == 0 and k == 0),
                                stop=False,
                            )

                    # tanh(enc_proj + dp) -> bf16
                    tbfs = []
                    for cc in range(AC):
                        targ = ssb.tile([128, BL, P], f32, tag="targ")
                        nc.vector.tensor_add(
                            targ, enc_projT[:, cc, :, :], bcast(dpT[:, cc, :], P)
                        )
                        tbf = ssb.tile([128, BL, P], bf16, tag=f"tbf{cc}")
                        nc.scalar.activation(out=tbf, in_=targ, func=Tanh)
                        tbfs.append(tbf)

                    # scores matvecs + exp (+denominator accumulation)
                    attw = ssb.tile([1, BL, P], f32, tag="attw")
                    denom = ssb.tile([1, BL], f32, tag="den")
                    for b in range(BL):
                        sc_ps = sp2.tile([1, P], f32, tag="sc")
                        for cc in range(AC):
                            nc.tensor.matmul(
                                sc_ps,
                                w_e_sb[:, cc:cc + 1],
                                tbfs[cc][:, b, :],
                                start=(cc == 0),
                                stop=(cc == AC - 1),
                            )
                        nc.scalar.activation(
                            out=attw[:, b, :], in_=sc_ps, func=Exp,
                            bias=b_e_sb[0:1, 0:1],
                            accum_out=denom[:, b:b + 1],
                        )
                    attw_bf = ssb.tile([1, BL, 256], bf16, tag="attwbf")
                    nc.vector.memset(attw_bf[:, :, P:256], 0.0)
                    nc.vector.tensor_copy(attw_bf[:, :, 0:P], attw)

                    # broadcast attw across partitions, context matvecs
                    ctx_ps = sp1.tile([128, HC, BL], f32, tag="ctx")
                    attP = ssb.tile([128, 2, BL], bf16, tag="attP")
                    for b in range(BL):
                        aw_ps = sp2.tile([128, 2], f32, tag="aw", bufs=1)
                        nc.tensor.matmul(
                            aw_ps[:, 0:1], attw_bf[:, b, 0:P0], one_bf,
                            start=True, stop=False,
                        )
                        nc.tensor.matmul(
                            aw_ps[:, 1:2], attw_bf[:, b, P0:256], one_bf,
                            start=False, stop=True,
                        )
                        nc.vector.tensor_copy(attP[:, :, b], aw_ps)

                    # context matvecs, chunk-major so gates can chase chunks
                    for m in range(HC):
                        for b in range(BL):
                            nc.tensor.matmul(
                                ctx_ps[:, m, b:b + 1],
                                encP_sb[:, 0, b, ts(m, 128)],
                                attP[:, 0, b:b + 1],
                                start=(m == 0 and b == 0),
                                stop=False,
                            )
                            nc.tensor.matmul(
                                ctx_ps[:, m, b:b + 1],
                                encP_sb[0:P1, 1, b, ts(m, 128)],
                                attP[0:P1, 1, b:b + 1],
                                start=False,
                                stop=(m == HC - 1 and b == BL - 1),
                            )

                    # reciprocal of denominators, broadcast to 128 partitions
                    rden = ssb.tile([1, BL], f32, tag="rden")
                    nc.vector.reciprocal(rden, denom)
                    rbc_ps = sp1.tile([128, BL], f32, tag="dp")
                    nc.tensor.matmul(rbc_ps, ones_f32, rden, start=True, stop=True)
                    rbc_sb = ssb.tile([128, BL], f32, tag="rbcsb")
                    nc.vector.tensor_copy(rbc_sb, rbc_ps)
                    # normalized context in bf16
                    ctx_bf = ssb.tile([128, HC, BL], bf16, tag="ctxbf")
                    nc.vector.tensor_mul(
                        ctx_bf, ctx_ps[:, 0:HC, :], bcast_mid(rbc_sb, HC)
                    )
                    if t == S - 1:
                        attw_n = ssb.tile([1, BL, P], f32, tag="attwn")
                        nc.vector.tensor_mul(attw_n, attw, bcast(rden, P))
                        nc.sync.dma_start(out=attw_o[0:1, :],
                                          in_=attw_n.rearrange("o b q -> o (b q)"))

                    # gates (ctx part), k-major to chase ctx chunks
                    for k in range(AC):
                        for m in range(GC):
                            nc.tensor.matmul(
                                g_ps[:, m, :],
                                WchT_sb[:, k, ts(m, 128)],
                                ctx_bf[:, k, :],
                                start=False,
                                stop=(m == GC - 1 and k == AC - 1),
                            )

                    # gates += emb_pre; LSTM cell update (gate order i,f,o,g)
                    gsb = ssb.tile([128, GC, BL], f32, tag="gsb")
                    nc.vector.tensor_add(
                        gsb, g_ps[:, 0:GC, :], emb_preT[:, :, ts(t, BL)]
                    )
                    tio = ssb.tile([128, 3 * HC, BL], f32, tag="tio")
                    nc.scalar.activation(
                        out=tio, in_=gsb[:, 0:3 * HC, :], func=Tanh, scale=0.5
                    )
                    tg = ssb.tile([128, HC, BL], f32, tag="tg")
                    nc.scalar.activation(
                        out=tg, in_=gsb[:, 3 * HC:4 * HC, :], func=Tanh
                    )
                    sif = ssb.tile([128, 2 * HC, BL], f32, tag="sif")
                    nc.vector.tensor_scalar(
                        sif, tio[:, 0:2 * HC, :], 1.0, 0.5, op0=ADD, op1=MULT
                    )
                    t1 = ssb.tile([128, HC, BL], f32, tag="t1")
                    nc.vector.tensor_mul(t1, sif[:, HC:2 * HC, :], c_st)
                    t2 = ssb.tile([128, HC, BL], f32, tag="t2")
                    nc.vector.tensor_mul(t2, sif[:, 0:HC, :], tg)
                    nc.vector.tensor_add(c_st, t1, t2)
                    tc_t = ssb.tile([128, HC, BL], f32, tag="tct")
                    nc.scalar.activation(out=tc_t, in_=c_st, func=Tanh)
                    # hist <- 2h = (tanh(o/2)+1) * tanh(c)
                    nc.vector.scalar_tensor_tensor(
                        hnxt, tio[:, 2 * HC:3 * HC, :], 1.0, tc_t,
                        op0=ADD, op1=MULT,
                    )

                    # pipeline: allgather finished groups, drip vocab work
                    for gi, (s0, sg) in enumerate(GROUPS):
                        if t == s0 + sg - 1:
                            emit_group_ag(gi)
                    if t >= 4:
                        emit_vocab_jobs(4)

                emit_vocab_jobs(len(vocab_jobs))

    nc.compile()
    return nc


def _prep_inputs(inputs):
    enc = np.asarray(inputs["encoder_outputs"], np.float32)      # [B, P, H]
    captions = np.asarray(inputs["captions"])
    embedding = np.asarray(inputs["embedding"], np.float32)      # [V, E]
    W_ih = np.asarray(inputs["W_ih"], np.float32)                # [4H, E+H]
    W_hh = np.asarray(inputs["W_hh"], np.float32)                # [4H, H]
    b_ih = np.asarray(inputs["b_ih"], np.float32)
    b_hh = np.asarray(inputs["b_hh"], np.float32)
    We = np.asarray(inputs["We"], np.float32)                    # [A, H]
    be = np.asarray(inputs["be"], np.float32)
    Wd = np.asarray(inputs["Wd"], np.float32)
    bd = np.asarray(inputs["bd"], np.float32)
    w_e = np.asarray(inputs["w_e"], np.float32)                  # [A]
    b_e = np.asarray(inputs["b_e"], np.float32)
    W_out = np.asarray(inputs["W_out"], np.float32)              # [V, H]

    emb = embedding[captions[:, :-1]]                            # [B, S, E]

    # permute gates from torch order [i, f, g, o] to [i, f, o, g]
    perm = np.concatenate([
        np.arange(0, H), np.arange(H, 2 * H),
        np.arange(3 * H, 4 * H), np.arange(2 * H, 3 * H),
    ])
    W_ih = W_ih[perm]
    W_hh = W_hh[perm]
    bias_g = (b_ih + b_hh)[perm]

    W_ie = W_ih[:, :E]
    W_ic = W_ih[:, E:]
    # hist stores 2h -> pre-scale every matrix that multiplies h by 0.5
    WchT = np.concatenate([W_ic.T, 0.5 * W_hh.T], axis=0)        # [2H, 4H]
    WieT = W_ie.T                                                # [E, 4H]
    bias_a = (be + bd).reshape(AC, 128).T                        # [128, AC]
    bias_g = bias_g.reshape(GC, 128).T                           # [128, GC]
    w_e_t = w_e.reshape(AC, 128).T                               # [128, AC]

    shared = {
        "WeT": _dt(We.T, BF16),
        "WdT": _dt(0.5 * Wd.T, BF16),
        "WchT": _dt(WchT, BF16),
        "WieT": _dt(WieT, BF16),
        "w_e": _dt(w_e_t, BF16),
        "bias_a": _dt(bias_a, np.float32),
        "bias_g": _dt(bias_g, np.float32),
        "b_e": _dt(b_e.reshape(1, 1), np.float32),
    }

    in_maps = []
    for j in range(NCORES):
        bsl = slice(j * BL, (j + 1) * BL)
        enc_j = enc[bsl]                                          # [BL, P, H]
        encT = enc_j.reshape(BL * P, H).T                         # [H, BL*P]
        encP = np.zeros((2, 128, BL, H), np.float32)
        encP[0] = enc_j.transpose(1, 0, 2)[0:128]
        encP[1, 0:P1] = enc_j.transpose(1, 0, 2)[128:P]
        embT = emb[bsl].transpose(2, 1, 0).reshape(E, R)          # [E, (t,b)]
        WoT = 0.5 * W_out[j * VL:(j + 1) * VL].T                  # [H, VL]
        m = dict(shared)
        m.update({
            "encT": _dt(encT, BF16),
            "encP": _dt(encP, BF16),
            "embT": _dt(embT, BF16),
            "WoT": _dt(WoT, BF16),
        })
        in_maps.append(m)
    return in_maps


def _assemble(res, b_out):
    outputs = np.zeros((B, T, V), np.float32)
    for j in range(NCORES):
        o = res[j]["out_v"]                                       # [ROWS, VL]
        r0 = 0
        for gi, (s0, sg) in enumerate(GROUPS):
            blk = o[r0:r0 + 4 * sg * NCORES].reshape(NCORES, sg, BL, VL)
            outputs[:, 1 + s0:1 + s0 + sg, j * VL:(j + 1) * VL] = (
                blk.transpose(0, 2, 1, 3).reshape(B, sg, VL)
            )
            r0 += 4 * sg * NCORES
    if np.any(b_out):
        outputs[:, 1:, :] += b_out
    attw = np.concatenate(
        [res[j]["attw_o"].reshape(BL, P) for j in range(NCORES)], axis=0
    )
    return outputs, attw


def kernel(**inputs):
    if "nc" not in _CACHED:
        _CACHED["nc"] = _build_program()
    nc = _CACHED["nc"]

    in_maps = _prep_inputs(inputs)
    res = run_bass_kernel_spmd(nc, in_maps, list(range(NCORES))).results
    return _assemble(res, np.asarray(inputs["b_out"], np.float32))
